# revision 9
# baseline (speedup 1.0000x reference)
"""Trainium2 Bass kernel for nn_A5ExactScan: B=16384 rows, T=2048-step table-lookup scan.

Per NeuronCore (8 cores, 2048 rows each), transposed one-hot representation:
per-step tensors are [120, NW] column-split into W independent row-wave chains.
Rows 0-1023 live on partitions 0-59 (block 0), rows 1024-2047 on partitions
60-119 (block 1); within a block, column i is row j*1024+i.

The token one-hot U[c+60j, i, t] = [x[row] == c] is precomputed on the host and
streamed from HBM (data-independent), so the per-step device work is only:
    PE  : G2 = MULS2.T @ U_t       (G2[c+60j, i] = mul[x_i, c])
    ACT : G2S = bf16(G2)           (PSUM -> SBUF, off critical chain)
    DVE : H = is_eq(SN, iota)*G2S  (select by state, SN = replicated state PSUM)
    PE  : SN' = ONES2.T @ H        (select-reduce + replicate next state)
Final: logitsT = 5.0 * is_eq(SN, iota) -> DRAM [120, 1024] fp32 per core.
"""
import sys
import numpy as np

sys.path.insert(0, "/opt/trn_rl_repo")

B, T = 16384, 2048
NS = 60          # number of states / tokens
NB = 1024        # rows per block
P2 = 120         # 2 blocks * 60 partitions
NCORES = 8
RPC = B // NCORES  # rows per core = 2048
CH = 16          # steps per DMA chunk
NWAVES = 2       # independent row-wave chains
NW = NB // NWAVES


def _build(t_steps: int):
    import concourse.bacc as bacc
    import concourse.mybir as mybir
    from concourse.tile import TileContext

    AL = mybir.AluOpType
    BF = mybir.dt.bfloat16
    F32 = mybir.dt.float32

    nc = bacc.Bacc("TRN2", num_devices=NCORES)
    u_in = nc.declare_dram_parameter("u", [P2, t_steps * NB], mybir.dt.float8e4, isOutput=False)
    muls_in = nc.declare_dram_parameter("muls", [P2, P2], BF, isOutput=False)
    iota_in = nc.declare_dram_parameter("iota", [P2, 1], F32, isOutput=False)
    ones2_in = nc.declare_dram_parameter("ones2", [P2, P2], BF, isOutput=False)
    lg_out = nc.declare_dram_parameter("logitsT", [P2, NB], F32, isOutput=True)

    n_chunks = (t_steps + CH - 1) // CH

    with TileContext(nc) as tc:
        with (
            tc.tile_pool(name="const", bufs=1) as cpool,
            tc.tile_pool(name="stage", bufs=2) as spool,
            tc.tile_pool(name="work", bufs=3) as wpool,
            tc.tile_pool(name="ps_g2", bufs=2, space="PSUM") as ps_g2,
            tc.tile_pool(name="ps_sn", bufs=2, space="PSUM") as ps_sn,
        ):
            muls = cpool.tile([P2, P2], BF)
            iota = cpool.tile([P2, 1], F32)
            ones2 = cpool.tile([P2, P2], BF)
            hz = cpool.tile([P2, NB], BF)
            lgf = cpool.tile([P2, NB], F32)

            nc.sync.dma_start(out=muls[:], in_=muls_in[:])
            nc.sync.dma_start(out=iota[:], in_=iota_in[:])
            nc.sync.dma_start(out=ones2[:], in_=ones2_in[:])
            nc.vector.memset(hz[:], 0.0)

            # initial state per wave: SN = ONES2.T @ 0 -> state 0 replicated
            sn = []
            for w in range(NWAVES):
                snw = ps_sn.tile([P2, NW], mybir.dt.float32, tag=f"sn{w}")
                nc.tensor.matmul(snw[:], ones2[:], hz[:, :NW])
                sn.append(snw)

            for ch in range(n_chunks):
                stage = spool.tile([P2, CH * NB], mybir.dt.float8e4, tag="stage")
                nc.sync.dma_start(
                    out=stage[:], in_=u_in[:, ch * CH * NB : (ch + 1) * CH * NB]
                )
                for u in range(CH):
                    t = ch * CH + u
                    if t >= t_steps:
                        break
                    for w in range(NWAVES):
                        g2 = ps_g2.tile([P2, NW], mybir.dt.float32, tag=f"g2{w}")
                        sn_next = ps_sn.tile([P2, NW], mybir.dt.float32, tag=f"sn{w}")
                        g2s = wpool.tile([P2, NW], BF, tag=f"g2s{w}")
                        h = wpool.tile([P2, NW], BF, tag=f"h{w}")
                        usl = stage[:, u * NB + w * NW : u * NB + (w + 1) * NW]
                        nc.tensor.matmul(g2[:], muls[:], usl)
                        nc.scalar.copy(out=g2s[:], in_=g2[:])
                        nc.vector.scalar_tensor_tensor(
                            out=h[:], in0=sn[w][:], scalar=iota[:], in1=g2s[:],
                            op0=AL.is_equal, op1=AL.mult,
                        )
                        nc.tensor.matmul(sn_next[:], ones2[:], h[:])
                        sn[w] = sn_next

            for w in range(NWAVES):
                nc.vector.tensor_scalar(
                    out=lgf[:, w * NW : (w + 1) * NW], in0=sn[w][:],
                    scalar1=iota[:], scalar2=5.0,
                    op0=AL.is_equal, op1=AL.mult,
                )
            nc.sync.dma_start(out=lg_out[:], in_=lgf[:])

    nc.compile()
    return nc


def _prep_inputs(input_ids: np.ndarray, mul: np.ndarray, t_steps: int):
    import ml_dtypes

    BF = ml_dtypes.bfloat16
    mul_f = mul.astype(np.float32)
    muls_np = np.zeros((P2, P2), np.float32)
    muls_np[:NS, :NS] = mul_f
    muls_np[NS:P2, NS:P2] = mul_f
    iota_np = (np.arange(P2) % NS).astype(np.float32).reshape(P2, 1)
    ones2_np = np.zeros((P2, P2), np.float32)
    ones2_np[:NS, :NS] = 1.0
    ones2_np[NS:, NS:] = 1.0

    consts = {
        "muls": muls_np.astype(BF),
        "iota": iota_np,
        "ones2": ones2_np.astype(BF),
    }

    iota60 = np.arange(NS, dtype=np.int32)
    in_maps = []
    for k in range(NCORES):
        shard = input_ids[k * RPC : (k + 1) * RPC, :t_steps].astype(np.int32)
        # xarr[j, t, i] = x[row j*NB+i, t]
        xarr = shard.reshape(2, NB, t_steps).transpose(0, 2, 1)  # [2, t, NB]
        # u[c + 60j, t*NB + i] = [xarr[j, t, i] == c], as fp8e4m3 bytes (1.0 = 0x38)
        u8 = (xarr[:, None, :, :] == iota60[None, :, None, None])  # [2, 60, t, NB] bool
        u = (u8.astype(np.uint8) * np.uint8(0x38)).reshape(P2, t_steps * NB)
        u = u.view(ml_dtypes.float8_e4m3fn)
        m = dict(consts)
        m["u"] = u
        in_maps.append(m)
    return in_maps


def kernel(input_ids: np.ndarray, mul: np.ndarray, t_steps: int | None = None) -> np.ndarray:
    from concourse.bass_utils import run_bass_kernel_spmd

    t_steps = T if t_steps is None else t_steps
    nc = _build(t_steps)
    in_maps = _prep_inputs(np.asarray(input_ids), np.asarray(mul), t_steps)
    res = run_bass_kernel_spmd(nc, in_maps, core_ids=list(range(NCORES)), trace=True)
    kernel.last_exec_ns = res.exec_time_ns

    logits = np.zeros((B, NS), np.float32)
    for k in range(NCORES):
        lgt = res.results[k]["logitsT"]  # [120, 1024]
        for j in range(2):
            blk = lgt[j * NS : (j + 1) * NS, :]  # [60, 1024]
            logits[k * RPC + j * NB : k * RPC + (j + 1) * NB, :] = blk.T
    return logits


kernel.last_exec_ns = None

if __name__ == "__main__":
    t_steps = int(sys.argv[1]) if len(sys.argv) > 1 else 64
    rng = np.random.default_rng(0)
    x = rng.integers(0, NS, (B, T)).astype(np.int32)
    mul = rng.integers(0, NS, (NS, NS)).astype(np.int32)
    import time

    t0 = time.time()
    out = kernel(x, mul, t_steps=t_steps)
    t1 = time.time()
    s = np.zeros(B, np.int64)
    for t in range(t_steps):
        s = mul[x[:, t], s]
    exp = np.zeros((B, NS), np.float32)
    exp[np.arange(B), s] = 5.0
    print("wall:", round(t1 - t0, 1), "exec_ns:", kernel.last_exec_ns,
          "per-step:", (kernel.last_exec_ns or 0) / t_steps)
    print("match:", np.array_equal(out, exp))


# revision 10
# speedup vs baseline: 1.0007x; 1.0007x over previous
"""Trainium2 Bass kernel for nn_A5ExactScan: B=16384 rows, T=2048-step table-lookup scan.

Per NeuronCore (8 cores, 2048 rows each), transposed one-hot representation:
per-step tensors are [120, NW] column-split into W independent row-wave chains.
Rows 0-1023 live on partitions 0-59 (block 0), rows 1024-2047 on partitions
60-119 (block 1); within a block, column i is row j*1024+i.

The token one-hot U[c+60j, i, t] = [x[row] == c] is precomputed on the host and
streamed from HBM (data-independent), so the per-step device work is only:
    PE  : G2 = MULS2.T @ U_t       (G2[c+60j, i] = mul[x_i, c])
    ACT : G2S = bf16(G2)           (PSUM -> SBUF, off critical chain)
    DVE : H = is_eq(SN, iota)*G2S  (select by state, SN = replicated state PSUM)
    PE  : SN' = ONES2.T @ H        (select-reduce + replicate next state)
Final: logitsT = 5.0 * is_eq(SN, iota) -> DRAM [120, 1024] fp32 per core.
"""
import sys
import numpy as np

sys.path.insert(0, "/opt/trn_rl_repo")

B, T = 16384, 2048
NS = 60          # number of states / tokens
NB = 1024        # rows per block
P2 = 120         # 2 blocks * 60 partitions
NCORES = 8
RPC = B // NCORES  # rows per core = 2048
CH = 16          # steps per DMA chunk
NWAVES = 2       # independent row-wave chains
NW = NB // NWAVES


def _build(t_steps: int):
    import concourse.bacc as bacc
    import concourse.mybir as mybir
    from concourse.tile import TileContext

    AL = mybir.AluOpType
    BF = mybir.dt.bfloat16
    F32 = mybir.dt.float32

    nc = bacc.Bacc("TRN2", num_devices=NCORES)
    u_in = nc.declare_dram_parameter("u", [P2, t_steps * NB], mybir.dt.float8e4, isOutput=False)
    muls_in = nc.declare_dram_parameter("muls", [P2, P2], BF, isOutput=False)
    iota_in = nc.declare_dram_parameter("iota", [P2, 1], F32, isOutput=False)
    ones2_in = nc.declare_dram_parameter("ones2", [P2, P2], BF, isOutput=False)
    lg_out = nc.declare_dram_parameter("logitsT", [P2, NB], F32, isOutput=True)

    n_chunks = (t_steps + CH - 1) // CH

    with TileContext(nc) as tc:
        with (
            tc.tile_pool(name="const", bufs=1) as cpool,
            tc.tile_pool(name="stage", bufs=2) as spool,
            tc.tile_pool(name="work", bufs=3) as wpool,
            tc.tile_pool(name="ps_g2", bufs=2, space="PSUM") as ps_g2,
            tc.tile_pool(name="ps_sn", bufs=2, space="PSUM") as ps_sn,
        ):
            muls = cpool.tile([P2, P2], BF)
            iota = cpool.tile([P2, 1], F32)
            ones2 = cpool.tile([P2, P2], BF)
            hz = cpool.tile([P2, NB], BF)
            lgf = cpool.tile([P2, NB], F32)

            nc.sync.dma_start(out=muls[:], in_=muls_in[:])
            nc.sync.dma_start(out=iota[:], in_=iota_in[:])
            nc.sync.dma_start(out=ones2[:], in_=ones2_in[:])
            nc.vector.memset(hz[:], 0.0)

            # initial state per wave: SN = ONES2.T @ 0 -> state 0 replicated
            sn = []
            for w in range(NWAVES):
                snw = ps_sn.tile([P2, NW], mybir.dt.float32, tag=f"sn{w}")
                nc.tensor.matmul(snw[:], ones2[:], hz[:, :NW])
                sn.append(snw)

            for ch in range(n_chunks):
                stage = spool.tile([P2, CH * NB], mybir.dt.float8e4, tag="stage")
                nc.sync.dma_start(
                    out=stage[:], in_=u_in[:, ch * CH * NB : (ch + 1) * CH * NB]
                )
                for u in range(CH):
                    t = ch * CH + u
                    if t >= t_steps:
                        break
                    for w in range(NWAVES):
                        g2 = ps_g2.tile([P2, NW], mybir.dt.float32, tag=f"g2{w}")
                        sn_next = ps_sn.tile([P2, NW], mybir.dt.float32, tag=f"sn{w}")
                        g2s = wpool.tile([P2, NW], BF, tag=f"g2s{w}")
                        h = wpool.tile([P2, NW], BF, tag=f"h{w}")
                        usl = stage[:, u * NB + w * NW : u * NB + (w + 1) * NW]
                        nc.tensor.matmul(g2[:], muls[:], usl)
                        nc.scalar.copy(out=g2s[:], in_=g2[:])
                        nc.vector.scalar_tensor_tensor(
                            out=h[:], in0=sn[w][:], scalar=iota[:], in1=g2s[:],
                            op0=AL.is_equal, op1=AL.mult,
                        )
                        nc.tensor.matmul(sn_next[:], ones2[:], h[:])
                        sn[w] = sn_next

            for w in range(NWAVES):
                nc.vector.tensor_scalar(
                    out=lgf[:, w * NW : (w + 1) * NW], in0=sn[w][:],
                    scalar1=iota[:], scalar2=5.0,
                    op0=AL.is_equal, op1=AL.mult,
                )
            nc.sync.dma_start(out=lg_out[:], in_=lgf[:])

    nc.compile()
    return nc


def _prep_inputs(input_ids: np.ndarray, mul: np.ndarray, t_steps: int):
    import ml_dtypes

    BF = ml_dtypes.bfloat16
    mul_f = mul.astype(np.float32)
    muls_np = np.zeros((P2, P2), np.float32)
    muls_np[:NS, :NS] = mul_f
    muls_np[NS:P2, NS:P2] = mul_f
    iota_np = (np.arange(P2) % NS).astype(np.float32).reshape(P2, 1)
    ones2_np = np.zeros((P2, P2), np.float32)
    ones2_np[:NS, :NS] = 1.0
    ones2_np[NS:, NS:] = 1.0

    consts = {
        "muls": muls_np.astype(BF),
        "iota": iota_np,
        "ones2": ones2_np.astype(BF),
    }

    iota60 = np.arange(NS, dtype=np.int32)
    in_maps = []
    for k in range(NCORES):
        shard = input_ids[k * RPC : (k + 1) * RPC, :t_steps].astype(np.int32)
        # xarr[j, t, i] = x[row j*NB+i, t]
        xarr = shard.reshape(2, NB, t_steps).transpose(0, 2, 1)  # [2, t, NB]
        # u[c + 60j, t*NB + i] = [xarr[j, t, i] == c], as fp8e4m3 bytes (1.0 = 0x38)
        u8 = (xarr[:, None, :, :] == iota60[None, :, None, None])  # [2, 60, t, NB] bool
        u = (u8.astype(np.uint8) * np.uint8(0x38)).reshape(P2, t_steps * NB)
        u = u.view(ml_dtypes.float8_e4m3fn)
        m = dict(consts)
        m["u"] = u
        in_maps.append(m)
    return in_maps


def _ensure_ntff_hook():
    """Register the axon NTFF profile hook if the image's antenv lacks it.

    run_bass_kernel_spmd(trace=True) under axon imports antenv.axon_hooks; on
    images where that module is missing, inject it and wire up the hook that
    trn_agent_boot would have registered at boot.
    """
    try:
        import antenv.axon_hooks  # noqa: F401
        return
    except ImportError:
        pass
    import types

    import antenv

    mod = types.ModuleType("antenv.axon_hooks")
    mod._h = None
    mod.set_axon_ntff_profile_hook = lambda h: setattr(mod, "_h", h)
    mod.get_axon_ntff_profile_hook = lambda: mod._h
    sys.modules["antenv.axon_hooks"] = mod
    antenv.axon_hooks = mod
    try:
        from trn_agent_boot.trn_boot import _ntff_profile_via_ctypes

        mod._h = _ntff_profile_via_ctypes("/opt/axon/libaxon_pjrt.so")
    except Exception:
        pass


def kernel(input_ids: np.ndarray, mul: np.ndarray, t_steps: int | None = None) -> np.ndarray:
    from concourse.bass_utils import run_bass_kernel_spmd

    t_steps = T if t_steps is None else t_steps
    nc = _build(t_steps)
    in_maps = _prep_inputs(np.asarray(input_ids), np.asarray(mul), t_steps)
    _ensure_ntff_hook()
    try:
        res = run_bass_kernel_spmd(nc, in_maps, core_ids=list(range(NCORES)), trace=True)
    except Exception:
        res = run_bass_kernel_spmd(nc, in_maps, core_ids=list(range(NCORES)), trace=False)
    kernel.last_exec_ns = res.exec_time_ns

    logits = np.zeros((B, NS), np.float32)
    for k in range(NCORES):
        lgt = res.results[k]["logitsT"]  # [120, 1024]
        for j in range(2):
            blk = lgt[j * NS : (j + 1) * NS, :]  # [60, 1024]
            logits[k * RPC + j * NB : k * RPC + (j + 1) * NB, :] = blk.T
    return logits


kernel.last_exec_ns = None

if __name__ == "__main__":
    t_steps = int(sys.argv[1]) if len(sys.argv) > 1 else 64
    rng = np.random.default_rng(0)
    x = rng.integers(0, NS, (B, T)).astype(np.int32)
    mul = rng.integers(0, NS, (NS, NS)).astype(np.int32)
    import time

    t0 = time.time()
    out = kernel(x, mul, t_steps=t_steps)
    t1 = time.time()
    s = np.zeros(B, np.int64)
    for t in range(t_steps):
        s = mul[x[:, t], s]
    exp = np.zeros((B, NS), np.float32)
    exp[np.arange(B), s] = 5.0
    print("wall:", round(t1 - t0, 1), "exec_ns:", kernel.last_exec_ns,
          "per-step:", (kernel.last_exec_ns or 0) / t_steps)
    print("match:", np.array_equal(out, exp))
